# revision 9
# baseline (speedup 1.0000x reference)
"""Two-layer GAT (N=4096, 4 heads, HID=256) on 8 TRN2 NeuronCores.

Sharding: each core owns N/8 = 512 destination rows of every N^2 attention
matrix. Weights are replicated. Per head we compute the local projection
g_shard = h_shard @ W.T on the owning core, then AllGather a packed
[512, 258] payload (g | ones | s_dst) so every core has the full
[4096, 258] g_aug for the attention matmul.

Softmax is computed without any row reductions: the masked exp matrix P
multiplies g_aug whose column 256 is all-ones, so the PSUM accumulator
holds both the numerator P@g and the denominator P@1; a per-partition
reciprocal-multiply normalizes after the matmul.

The per-entry exp(leakyrelu(s_i + t_j)) is produced via the identity
exp(lrelu(x)) = max(exp(x), exp(alpha*x)); both branches factor into
rank-1 outer products of precomputed exponentials:
  u_i = exp(s_src_i), u'_i = exp(alpha*s_src_i)   (broadcast tiles)
  v_j = exp(s_dst_j), w_j = exp((alpha-1)*s_dst_j) (per-partition scalars)
  pm[j,i] = mask * v_j * max(u'_i * w_j, u_i)
which is two chained scalar_tensor_tensor ops per [128, 512] tile, all
bf16 (2x DVE mode). A third of the tiles instead use the ACT engine
(Prelu+Exp as in the direct formulation) purely for engine balance, and
some mask multiplies go to GPSIMD; the recipe keeps DVE/ACT/GPSIMD all
below the PE's tile consumption rate.

Layout choice: attention tiles are [j=source (partition), i=dest (free)],
so P tiles feed the PE matmul directly as lhsT with no transposes.

All matmul operands are bf16 (fp32 matmuls run at 4 cycles/row on TRN2 vs
1 for bf16); accumulation stays fp32 in PSUM.

Stall avoidance: the 4 MB adjacency mask streams in on the vector/gpsimd
DMA queues so it cannot delay the weight/feature loads (sync/scalar
queues) that gate the first projections; head 0's AllGather is split in
two so attention starts after half the payload has arrived.
"""

import os

import numpy as np
import ml_dtypes

import concourse.bass as bass
import concourse.tile as tile
from concourse import bacc, mybir
from concourse.bass_utils import run_bass_kernel_spmd
from concourse.masks import make_identity

N, IN_DIM, HID, HEADS, OUT_DIM = 4096, 768, 256, 4, 32
ALPHA = 0.2
NCORES = 8
R = N // NCORES          # 512 rows per core
RB = R // 128            # 4 row blocks
FB = IN_DIM // 128       # 6 feature blocks
JB = N // 128            # 32 source chunks
HC = (HID * HEADS) // 128  # 8 concat-feature chunks
GW = HID + 2             # payload width: g (256) | ones | s_dst

F32 = mybir.dt.float32
BF16 = mybir.dt.bfloat16
AF = mybir.ActivationFunctionType
OP = mybir.AluOpType

last_exec_time_ns = None
_nc_cache = None


def _tile_recipe(k):
    """Returns (kind, mask_eng): kind 'A' = ACT Prelu+Exp path with the mask
    multiply on gpsimd (tensor_tensor is the only elementwise op its ISA
    supports), 'V' = rank-1 two-stt path, entirely on the vector engine."""
    if k % 3 == 0:
        return "A", ("D" if k == 0 else "G")
    return "V", None


def _build_layer(nc, tc, pools, x_tiles, W_ap, WT_ap, ap_ap, mask_all, L):
    """One GAT layer. x_tiles: 6 SBUF tiles [128, R] bf16 (features x rows,
    feature-major). Returns 8 SBUF tiles [128, R] bf16 = concat-head
    activations transposed (x_gatT), elu applied."""
    sb = pools["sb"]
    ps_acc = pools["ps_acc"]
    ps_big = pools["ps_big"]
    ps_sm = pools["ps_sm"]
    dram_pay = pools["dram_pay"]
    dram_gat = pools["dram_gat"]
    ones_row = pools["ones_row"]
    ident = pools["ident"]

    groups = [list(range(NCORES))]

    head_state = []
    # ---- Phase A: per-head projection + payload + AllGather ----
    for h in range(HEADS):
        # weights for this head
        W_t = []
        for cc in range(2):
            wt = sb.tile([128, IN_DIM], BF16, name=f"W_L{L}h{h}c{cc}", tag="Wh", bufs=4)
            nc.sync.dma_start(out=wt[:, :], in_=W_ap[h, cc * 128:(cc + 1) * 128, :])
            W_t.append(wt)
        WTaug = []
        for fb in range(FB):
            wta = sb.tile([128, HID + 1], BF16, name=f"WTa_L{L}h{h}f{fb}", tag="WTaug",
                          bufs=2 * FB)
            nc.sync.dma_start(out=wta[:, 0:HID], in_=WT_ap[h, fb * 128:(fb + 1) * 128, :])
            WTaug.append(wta)
        a_t = []
        for cc in range(2):
            at = sb.tile([128, 2], BF16, name=f"a_L{L}h{h}c{cc}", tag="ah", bufs=4)
            nc.sync.dma_start(out=at[:, :], in_=ap_ap[h, cc * 128:(cc + 1) * 128, :])
            a_t.append(at)

        # w_eff[f, 0:2] = W.T @ [a_src | a_dst]  -> [768, 2] in 6 blocks
        weff = []
        for fb in range(FB):
            pw = ps_sm.tile([128, 2], F32, name=f"pw_L{L}h{h}f{fb}", tag="ps_sm")
            for cc in range(2):
                nc.tensor.matmul(pw[:, :], lhsT=W_t[cc][:, fb * 128:(fb + 1) * 128],
                                 rhs=a_t[cc][:, :], start=(cc == 0), stop=(cc == 1))
            wf = sb.tile([128, 2], BF16, name=f"weff_L{L}h{h}f{fb}", tag="weff",
                         bufs=2 * FB)
            nc.vector.tensor_copy(wf[:, :], pw[:, :])
            # dst half becomes column HID of the projection rhs
            nc.vector.tensor_copy(WTaug[fb][:, HID:HID + 1], wf[:, 1:2])
            weff.append(wf)

        # s_srcT [1, R] = w_eff_src.T @ x
        ps_s = ps_sm.tile([1, R], F32, name=f"ps_s_L{L}h{h}", tag="ps_sm")
        for fb in range(FB):
            nc.tensor.matmul(ps_s[:, :], lhsT=weff[fb][:, 0:1], rhs=x_tiles[fb][:, :],
                             start=(fb == 0), stop=(fb == FB - 1))
        ssrcT = sb.tile([1, R], BF16, name=f"ssrcT_L{L}h{h}", tag="ssrcT", bufs=2)
        nc.vector.tensor_copy(ssrcT[:, :], ps_s[:, :])

        # broadcast s_src across partitions: [128, R] fp32 in PSUM, then
        # derive sbc (fp32, feeds ACT Prelu), u_b = exp(s), u'_b = exp(a*s)
        pb = ps_big.tile([128, R], F32, name=f"pb_L{L}h{h}", tag="ps_big")
        nc.tensor.matmul(pb[:, :], lhsT=ones_row[0:1, :], rhs=ssrcT[:, :],
                         start=True, stop=True)
        sbc = sb.tile([128, R], F32, name=f"sbc_L{L}h{h}", tag="sbc", bufs=2)
        nc.scalar.copy(sbc[:, :], pb[:, :])
        ub = sb.tile([128, R], BF16, name=f"ub_L{L}h{h}", tag="ub", bufs=2)
        nc.scalar.activation(ub[:, :], pb[:, :], AF.Exp)
        upb = sb.tile([128, R], BF16, name=f"upb_L{L}h{h}", tag="upb", bufs=2)
        nc.scalar.activation(upb[:, :], pb[:, :], AF.Exp, scale=ALPHA)

        # g_aug = x.T @ WTaug -> [512, 257] (g | s_dst), packed to payload bf16
        pay_t = dram_pay.tile([R, GW], BF16, name=f"pay_L{L}h{h}", tag="pay")
        pl = sb.tile([128, RB * GW], BF16, name=f"pl_L{L}h{h}", tag="pl", bufs=2)
        for ib in range(RB):
            pg = ps_big.tile([128, HID + 1], F32, name=f"pg_L{L}h{h}b{ib}", tag="ps_big")
            for fb in range(FB):
                nc.tensor.matmul(pg[:, :], lhsT=x_tiles[fb][:, ib * 128:(ib + 1) * 128],
                                 rhs=WTaug[fb][:, :], start=(fb == 0),
                                 stop=(fb == FB - 1))
            o = ib * GW
            if ib % 2 == 0:
                nc.vector.tensor_copy(pl[:, o:o + HID], pg[:, 0:HID])
                nc.vector.tensor_copy(pl[:, o + HID + 1:o + HID + 2],
                                      pg[:, HID:HID + 1])
            else:
                nc.scalar.copy(pl[:, o:o + HID], pg[:, 0:HID])
                nc.scalar.copy(pl[:, o + HID + 1:o + HID + 2],
                               pg[:, HID:HID + 1])
            nc.vector.memset(pl[:, o + HID:o + HID + 1], 1.0)
        # one DMA: SBUF [p, (ib, c)] -> DRAM [(ib, p), c]
        nc.sync.dma_start(out=pay_t.rearrange("(ib p) c -> p ib c", p=128),
                          in_=pl.rearrange("p (ib c) -> p ib c", c=GW))

        if h == 0:
            # split gather: halves land earlier so attention starts sooner
            gat_a = dram_gat.tile([N // 2, GW], BF16, name=f"gatA_L{L}h{h}",
                                  tag="gat", addr_space="Shared")
            gat_b = dram_gat.tile([N // 2, GW], BF16, name=f"gatB_L{L}h{h}",
                                  tag="gat", addr_space="Shared")
            nc.gpsimd.collective_compute(
                "AllGather", OP.bypass, replica_groups=groups,
                ins=[pay_t[0:R // 2, :].opt()], outs=[gat_a.opt()],
            )
            nc.gpsimd.collective_compute(
                "AllGather", OP.bypass, replica_groups=groups,
                ins=[pay_t[R // 2:R, :].opt()], outs=[gat_b.opt()],
            )
            head_state.append(((gat_a, gat_b), sbc, ub, upb))
        else:
            gat_t = dram_gat.tile([N, GW], BF16, name=f"gat_L{L}h{h}", tag="gat",
                                  addr_space="Shared")
            nc.gpsimd.collective_compute(
                "AllGather", OP.bypass, replica_groups=groups,
                ins=[pay_t.opt()], outs=[gat_t.opt()],
            )
            head_state.append((gat_t, sbc, ub, upb))

    # ---- Phase B: attention per head ----
    xgatT = []
    for hc in range(HC):
        xg = sb.tile([128, R], BF16, name=f"xgatT_L{L}c{hc}", tag="xgatT", bufs=HC)
        xgatT.append(xg)

    for h in range(HEADS):
        gat_t, sbc, ub, upb = head_state[h]
        gf = sb.tile([128, JB * GW], BF16, name=f"gf_L{L}h{h}", tag="gf", bufs=2)
        gfr = gf.rearrange("p (j c) -> p j c", c=GW)
        if h == 0:
            # gf holds chunks in ARRIVAL order: position k = 16q + 8l + core
            # maps to global j-chunk 4*core + 2q + l. Only the adjacency mask
            # is indexed by global j; everything else uses position k.
            gat_a, gat_b = gat_t
            gfr4 = gf.rearrange("p (s cc c) -> p s cc c", s=4, c=GW)
            jorder = []
            for q, g_t in ((0, gat_a), (1, gat_b)):
                gr2 = g_t.rearrange("(cc l p) c -> p l cc c", p=128, l=2)
                for l in range(2):
                    s = 2 * q + l
                    eng = nc.sync if l == 0 else nc.scalar
                    eng.dma_start(out=gfr4[:, s:s + 1], in_=gr2[:, l:l + 1])
                    jorder.extend(4 * c + 2 * q + l for c in range(NCORES))
        else:
            for q in range(4):
                js = slice(q * JB // 4, (q + 1) * JB // 4)
                eng = nc.sync if q % 2 == 0 else nc.scalar
                eng.dma_start(out=gfr[:, js],
                              in_=gat_t.rearrange("(j p) c -> p j c", p=128)[:, js])
            jorder = list(range(JB))
        # all 32 s_dst columns (stride GW) as one fp32 strip: STT scalars and
        # ACT bias must be fp32
        sdst = sb.tile([128, JB], F32, name=f"sdst_L{L}h{h}", tag="sdst", bufs=2)
        for q in range(4):
            js = slice(q * JB // 4, (q + 1) * JB // 4)
            nc.vector.tensor_copy(sdst[:, js], gfr[:, js, GW - 1])
        # v = exp(s_dst), w = exp((alpha-1)*s_dst) strips (fp32 scalars)
        vst = sb.tile([128, JB], F32, name=f"vst_L{L}h{h}", tag="vst", bufs=2)
        nc.scalar.activation(vst[:, :], sdst[:, :], AF.Exp)
        wst = sb.tile([128, JB], F32, name=f"wst_L{L}h{h}", tag="wst", bufs=2)
        nc.scalar.activation(wst[:, :], sdst[:, :], AF.Exp, scale=ALPHA - 1.0)

        U = []
        for ib in range(RB):
            u = ps_acc.tile([128, HID + 1], F32, name=f"U_L{L}h{h}b{ib}", tag="ps_acc")
            U.append(u)

        mmult_all = mask_all
        for k, jj in enumerate(jorder):
            o = k * GW
            kind, meng = _tile_recipe(k)
            pm = sb.tile([128, R], BF16, name=f"pm_L{L}h{h}j{k}", tag="pm",
                         bufs=4)
            if kind == "A":
                # ACT path: lrelu (bias fused) then exp; DVE applies mask
                et = sb.tile([128, R], F32, name=f"et_L{L}h{h}j{k}", tag="et",
                             bufs=3)
                nc.scalar.activation(et[:, :], sbc[:, :], AF.Prelu,
                                     bias=sdst[:, k:k + 1], scale=1.0,
                                     alpha=ALPHA)
                p = sb.tile([128, R], BF16, name=f"p_L{L}h{h}j{k}", tag="p",
                            bufs=3)
                nc.scalar.activation(p[:, :], et[:, :], AF.Exp)
                eng = nc.gpsimd if meng == "G" else nc.vector
                eng.tensor_mul(pm[:, :], p[:, :],
                               mmult_all[:, jj * R:(jj + 1) * R])
            else:
                # rank-1 path: pm = mask * v_j * max(u'_i * w_j, u_i)
                t = sb.tile([128, R], BF16, name=f"t_L{L}h{h}j{k}", tag="t",
                            bufs=4)
                nc.vector.scalar_tensor_tensor(t[:, :], upb[:, :],
                                               wst[:, k:k + 1], ub[:, :],
                                               OP.mult, OP.max)
                nc.vector.scalar_tensor_tensor(pm[:, :], t[:, :],
                                               vst[:, k:k + 1],
                                               mmult_all[:, jj * R:(jj + 1) * R],
                                               OP.mult, OP.mult)
            for ib in range(RB):
                nc.tensor.matmul(U[ib][:, :], lhsT=pm[:, ib * 128:(ib + 1) * 128],
                                 rhs=gf[:, o:o + HID + 1], start=(k == 0),
                                 stop=(k == JB - 1))

        for ib in range(RB):
            rcp = sb.tile([128, 1], F32, name=f"rcp_L{L}h{h}b{ib}", tag="rcp", bufs=2)
            nc.vector.reciprocal(rcp[:, :], U[ib][:, HID:HID + 1])
            hn = sb.tile([128, HID], F32, name=f"hn_L{L}h{h}b{ib}", tag="hn", bufs=2)
            nc.vector.tensor_scalar(hn[:, :], U[ib][:, 0:HID], rcp[:, 0:1], None,
                                    OP.mult)
            # elu(x) = max(x, exp(min(x, 0)) - 1)
            t1 = sb.tile([128, HID], F32, name=f"t1_L{L}h{h}b{ib}", tag="t1", bufs=2)
            nc.vector.tensor_scalar_min(t1[:, :], hn[:, :], 0.0)
            t2 = sb.tile([128, HID], F32, name=f"t2_L{L}h{h}b{ib}", tag="t2", bufs=2)
            nc.scalar.activation(t2[:, :], t1[:, :], AF.Exp)
            eo = sb.tile([128, HID], BF16, name=f"eo_L{L}h{h}b{ib}", tag="eo", bufs=2)
            nc.vector.scalar_tensor_tensor(eo[:, :], t2[:, :], -1.0, hn[:, :],
                                           OP.add, OP.max)
            for cb in range(2):
                pt = ps_sm.tile([128, 128], BF16, name=f"pt_L{L}h{h}b{ib}c{cb}",
                                tag="ps_sm")
                nc.tensor.transpose(pt[:, :], eo[:, cb * 128:(cb + 1) * 128],
                                    ident[:, :])
                if cb == 0:
                    nc.vector.tensor_copy(
                        xgatT[h * 2 + cb][:, ib * 128:(ib + 1) * 128], pt[:, :])
                else:
                    nc.scalar.copy(
                        xgatT[h * 2 + cb][:, ib * 128:(ib + 1) * 128], pt[:, :])
    return xgatT


def _build_program():
    nc = bacc.Bacc("TRN2", target_bir_lowering=False, debug=False,
                   num_devices=NCORES)

    xT_in = nc.dram_tensor("xT", [IN_DIM, R], BF16, kind="ExternalInput").ap()
    mask_in = nc.dram_tensor("mask", [N, R], BF16, kind="ExternalInput").ap()
    W1_in = nc.dram_tensor("W1", [HEADS, HID, IN_DIM], BF16, kind="ExternalInput").ap()
    W1T_in = nc.dram_tensor("W1T", [HEADS, IN_DIM, HID], BF16, kind="ExternalInput").ap()
    a1_in = nc.dram_tensor("a1p", [HEADS, HID, 2], BF16, kind="ExternalInput").ap()
    W2_in = nc.dram_tensor("W2", [HEADS, HID, IN_DIM], BF16, kind="ExternalInput").ap()
    W2T_in = nc.dram_tensor("W2T", [HEADS, IN_DIM, HID], BF16, kind="ExternalInput").ap()
    a2_in = nc.dram_tensor("a2p", [HEADS, HID, 2], BF16, kind="ExternalInput").ap()
    outwT_in = nc.dram_tensor("outwT", [HID * HEADS, IN_DIM], BF16,
                              kind="ExternalInput").ap()
    outb_in = nc.dram_tensor("outb", [IN_DIM, 1], F32, kind="ExternalInput").ap()
    out2wT_in = nc.dram_tensor("out2wT", [HID * HEADS, OUT_DIM], BF16,
                               kind="ExternalInput").ap()
    out2b_in = nc.dram_tensor("out2b", [OUT_DIM, 1], F32, kind="ExternalInput").ap()
    outT = nc.dram_tensor("outT", [OUT_DIM, R], F32, kind="ExternalOutput").ap()

    with tile.TileContext(nc) as tc:
        with tc.tile_pool(name="sb", bufs=1) as sb, \
             tc.tile_pool(name="ps_acc", bufs=RB, space="PSUM") as ps_acc, \
             tc.tile_pool(name="ps_big", bufs=2, space="PSUM") as ps_big, \
             tc.tile_pool(name="ps_sm", bufs=2, space="PSUM") as ps_sm, \
             tc.tile_pool(name="dram_pay", bufs=4, space="DRAM") as dram_pay, \
             tc.tile_pool(name="dram_gat", bufs=4, space="DRAM") as dram_gat:

            pools = dict(sb=sb, ps_acc=ps_acc, ps_big=ps_big, ps_sm=ps_sm,
                         dram_pay=dram_pay, dram_gat=dram_gat)

            # constants (identity uses gpsimd affine_select - after the
            # collective so it does not delay the barrier)
            ident = sb.tile([128, 128], BF16, name="ident", tag="ident", bufs=1)
            make_identity(nc, ident[:, :])
            ones_row = sb.tile([1, 128], BF16, name="ones_row", tag="ones_row", bufs=1)
            nc.vector.memset(ones_row[:, :], 1.0)
            pools["ident"] = ident
            pools["ones_row"] = ones_row

            # critical-path inputs on the sync queue (front of the line)
            x0 = []
            for fb in range(FB):
                x = sb.tile([128, R], BF16, name=f"x0_{fb}", tag="x0", bufs=FB)
                nc.sync.dma_start(out=x[:, :], in_=xT_in[fb * 128:(fb + 1) * 128, :])
                x0.append(x)

            # adjacency mask (multiplicative 0/1) streams on the gpsimd queue
            # so it never delays weights or gathered features (only gpsimd /
            # sync / scalar can issue DMAs)
            mask_all = sb.tile([128, JB * R], BF16, name="mask_all", tag="mask",
                               bufs=1)
            for q in range(4):
                js = slice(q * JB // 4, (q + 1) * JB // 4)
                nc.gpsimd.dma_start(
                    out=mask_all.rearrange("p (j c) -> p j c", c=R)[:, js],
                    in_=mask_in.rearrange("(j p) c -> p j c", p=128)[:, js])

            # non-urgent weights ride the gpsimd queue behind the mask
            outw_t = []
            for hc in range(HC):
                w = sb.tile([128, IN_DIM], BF16, name=f"outw{hc}", tag="outw", bufs=HC)
                nc.gpsimd.dma_start(out=w[:, :],
                                    in_=outwT_in[hc * 128:(hc + 1) * 128, :])
                outw_t.append(w)
            out2w_t = []
            for hc in range(HC):
                w = sb.tile([128, OUT_DIM], BF16, name=f"out2w{hc}", tag="out2w",
                            bufs=HC)
                nc.gpsimd.dma_start(out=w[:, :],
                                    in_=out2wT_in[hc * 128:(hc + 1) * 128, :])
                out2w_t.append(w)
            outb_t = []
            for fb in range(FB):
                b = sb.tile([128, 1], F32, name=f"outb{fb}", tag="outb", bufs=FB)
                nc.gpsimd.dma_start(out=b[:, :], in_=outb_in[fb * 128:(fb + 1) * 128, :])
                outb_t.append(b)
            out2b_t = sb.tile([OUT_DIM, 1], F32, name="out2b", tag="out2b", bufs=1)
            nc.gpsimd.dma_start(out=out2b_t[:, :], in_=out2b_in[:, :])

            # ---- layer 1 ----
            xg1 = _build_layer(nc, tc, pools, x0, W1_in, W1T_in, a1_in, mask_all, 1)
            x1 = []
            for fb in range(FB):
                px = ps_big.tile([128, R], F32, name=f"px1_{fb}", tag="ps_big")
                for hc in range(HC):
                    nc.tensor.matmul(px[:, :], lhsT=outw_t[hc][:, fb * 128:(fb + 1) * 128],
                                     rhs=xg1[hc][:, :], start=(hc == 0),
                                     stop=(hc == HC - 1))
                x = sb.tile([128, R], BF16, name=f"x1_{fb}", tag="x1", bufs=FB)
                if fb % 2 == 0:
                    nc.vector.tensor_scalar(x[:, :], px[:, :], outb_t[fb][:, 0:1],
                                            None, OP.add)
                else:
                    nc.scalar.activation(x[:, :], px[:, :], AF.Identity,
                                         bias=outb_t[fb][:, 0:1])
                x1.append(x)

            # ---- layer 2 ----
            xg2 = _build_layer(nc, tc, pools, x1, W2_in, W2T_in, a2_in, mask_all, 2)
            po = ps_big.tile([OUT_DIM, R], F32, name="po", tag="ps_big")
            for hc in range(HC):
                nc.tensor.matmul(po[:, :], lhsT=out2w_t[hc][:, 0:OUT_DIM],
                                 rhs=xg2[hc][:, :], start=(hc == 0),
                                 stop=(hc == HC - 1))
            ot = sb.tile([OUT_DIM, R], F32, name="ot", tag="ot", bufs=1)
            nc.vector.tensor_scalar(ot[:, :], po[:, :], out2b_t[:, 0:1], None, OP.add)
            nc.sync.dma_start(out=outT[:, :], in_=ot[:, :])

    nc.compile()
    return nc


def _host_shards(label_mat, W1, a1, W2, a2, out_w, out_b, out2_w, out2_b, adj):
    f32 = np.float32
    bf16 = ml_dtypes.bfloat16
    label_T = np.asarray(label_mat, f32).T.astype(bf16)                 # [768, N]
    adjT = np.asarray(adj).T.astype(bf16)                               # [N, N]
    common = dict(
        W1=np.ascontiguousarray(np.asarray(W1, f32).astype(bf16)),
        W1T=np.ascontiguousarray(np.asarray(W1, f32).transpose(0, 2, 1).astype(bf16)),
        a1p=np.ascontiguousarray(np.asarray(a1, f32).reshape(HEADS, 2, HID)
                                 .transpose(0, 2, 1).astype(bf16)),
        W2=np.ascontiguousarray(np.asarray(W2, f32).astype(bf16)),
        W2T=np.ascontiguousarray(np.asarray(W2, f32).transpose(0, 2, 1).astype(bf16)),
        a2p=np.ascontiguousarray(np.asarray(a2, f32).reshape(HEADS, 2, HID)
                                 .transpose(0, 2, 1).astype(bf16)),
        outwT=np.ascontiguousarray(np.asarray(out_w, f32).T.astype(bf16)),
        outb=np.ascontiguousarray(np.asarray(out_b, f32).reshape(IN_DIM, 1)),
        out2wT=np.ascontiguousarray(np.asarray(out2_w, f32).T.astype(bf16)),
        out2b=np.ascontiguousarray(np.asarray(out2_b, f32).reshape(OUT_DIM, 1)),
    )
    in_maps = []
    for c in range(NCORES):
        sl = slice(c * R, (c + 1) * R)
        m = dict(common)
        m["xT"] = np.ascontiguousarray(label_T[:, sl])
        m["mask"] = np.ascontiguousarray(adjT[:, sl])
        in_maps.append(m)
    return in_maps


def kernel(**inputs):
    global _nc_cache, last_exec_time_ns
    if _nc_cache is None:
        _nc_cache = _build_program()
    nc = _nc_cache
    in_maps = _host_shards(**inputs)
    trace = os.environ.get("GAT_TRACE", "0") == "1"
    res = run_bass_kernel_spmd(nc, in_maps, list(range(NCORES)), trace=trace)
    last_exec_time_ns = res.exec_time_ns
    out = np.empty((N, OUT_DIM), np.float32)
    for c in range(NCORES):
        out[c * R:(c + 1) * R, :] = np.asarray(res.results[c]["outT"]).T
    return out


# revision 10
# speedup vs baseline: 1.3438x; 1.3438x over previous
"""Two-layer GAT (N=4096, 4 heads, HID=256) on 8 TRN2 NeuronCores.

Sharding: each core owns N/8 = 512 destination rows of every N^2 attention
matrix. Weights are replicated. Per head we compute the local projection
g_shard = h_shard @ W.T on the owning core, then AllGather a packed
[512, 258] payload so every core has the full g for the attention matmul.

Softmax restructuring (the key to engine balance):
  exp(lrelu(x)) = max(exp(x), exp(a*x)) with x = s_i + t_j, so the masked
  unnormalized weight is  m * v_j * u_i * max(rho_i * w_j, 1)  with
  u = exp(s_src), v = exp(s_dst), rho = exp((a-1)s_src), w = exp((a-1)s_dst).
  - u_i is constant along j, so it cancels in the softmax normalization:
    never computed.
  - v_j is folded into the gathered payload on the OWNING core: the payload
    carries [g*v | v | w], so the attention matmul against columns [0:257]
    directly accumulates both the numerator and the denominator of the
    v-weighted softmax.
  - what remains per [128, 512] attention tile is
        pm = (rho_bcast * w_j  max  1) * mask
    i.e. ONE tensor_scalar (4x DVE mode: single-source bf16) plus a mask
    tensor_tensor which is batched over FOUR j-chunks at a time
    ([128, 2048]) to amortize instruction overhead. The adjacency mask is
    permuted host-side into gather-arrival order so those 4-chunk slices
    are contiguous and the same layout works for the split first gather.

Layout: attention tiles are [j=source (partition), i=dest (free)], so pm
tiles feed the PE matmul directly as lhsT; accumulation in fp32 PSUM with
the denominator riding in column 256. All matmul operands bf16.

Stall avoidance: a tiny warm-up AllGather is issued first so the
cross-core entry barrier (which absorbs core launch skew) overlaps the
initial weight DMAs; the 4 MB adjacency mask streams on the gpsimd DMA
queue; head 0's real gather is split in two so attention starts after
half the payload has arrived.
"""

import os

import numpy as np
import ml_dtypes

import concourse.bass as bass
import concourse.tile as tile
from concourse import bacc, mybir
from concourse.bass_utils import run_bass_kernel_spmd
from concourse.masks import make_identity

N, IN_DIM, HID, HEADS, OUT_DIM = 4096, 768, 256, 4, 32
ALPHA = 0.2
NCORES = 8
R = N // NCORES          # 512 rows per core
RB = R // 128            # 4 row blocks
FB = IN_DIM // 128       # 6 feature blocks
JB = N // 128            # 32 source chunks
QG = 4                   # j-chunks per mask-multiply group
HC = (HID * HEADS) // 128  # 8 concat-feature chunks
GW = HID + 2             # payload width: g*v (256) | v | w

F32 = mybir.dt.float32
BF16 = mybir.dt.bfloat16
AF = mybir.ActivationFunctionType
OP = mybir.AluOpType

last_exec_time_ns = None
_nc_cache = None


def _build_layer(nc, tc, pools, x_tiles, W_ap, WT_ap, ap_ap, mask_all, L):
    """One GAT layer. x_tiles: 6 SBUF tiles [128, R] bf16 (features x rows,
    feature-major). Returns 8 SBUF tiles [128, R] bf16 = concat-head
    activations transposed (x_gatT), elu applied."""
    sb = pools["sb"]
    ps_acc = pools["ps_acc"]
    ps_big = pools["ps_big"]
    ps_sm = pools["ps_sm"]
    dram_pay = pools["dram_pay"]
    dram_gat = pools["dram_gat"]
    ones_row = pools["ones_row"]
    ident = pools["ident"]

    groups = [list(range(NCORES))]

    head_state = []
    # ---- Phase A: per-head projection + payload + AllGather ----
    for h in range(HEADS):
        # weights for this head
        W_t = []
        for cc in range(2):
            wt = sb.tile([128, IN_DIM], BF16, name=f"W_L{L}h{h}c{cc}", tag="Wh", bufs=4)
            nc.sync.dma_start(out=wt[:, :], in_=W_ap[h, cc * 128:(cc + 1) * 128, :])
            W_t.append(wt)
        WTaug = []
        for fb in range(FB):
            wta = sb.tile([128, HID + 1], BF16, name=f"WTa_L{L}h{h}f{fb}", tag="WTaug",
                          bufs=2 * FB)
            nc.sync.dma_start(out=wta[:, 0:HID], in_=WT_ap[h, fb * 128:(fb + 1) * 128, :])
            WTaug.append(wta)
        a_t = []
        for cc in range(2):
            at = sb.tile([128, 2], BF16, name=f"a_L{L}h{h}c{cc}", tag="ah", bufs=4)
            nc.sync.dma_start(out=at[:, :], in_=ap_ap[h, cc * 128:(cc + 1) * 128, :])
            a_t.append(at)

        # w_eff[f, 0:2] = W.T @ [a_src | a_dst]  -> [768, 2] in 6 blocks
        weff = []
        for fb in range(FB):
            pw = ps_sm.tile([128, 2], F32, name=f"pw_L{L}h{h}f{fb}", tag="ps_sm")
            for cc in range(2):
                nc.tensor.matmul(pw[:, :], lhsT=W_t[cc][:, fb * 128:(fb + 1) * 128],
                                 rhs=a_t[cc][:, :], start=(cc == 0), stop=(cc == 1))
            wf = sb.tile([128, 2], BF16, name=f"weff_L{L}h{h}f{fb}", tag="weff",
                         bufs=2 * FB)
            nc.vector.tensor_copy(wf[:, :], pw[:, :])
            # dst half becomes column HID of the projection rhs
            nc.vector.tensor_copy(WTaug[fb][:, HID:HID + 1], wf[:, 1:2])
            weff.append(wf)

        # s_srcT [1, R] = w_eff_src.T @ x
        ps_s = ps_sm.tile([1, R], F32, name=f"ps_s_L{L}h{h}", tag="ps_sm")
        for fb in range(FB):
            nc.tensor.matmul(ps_s[:, :], lhsT=weff[fb][:, 0:1], rhs=x_tiles[fb][:, :],
                             start=(fb == 0), stop=(fb == FB - 1))
        ssrcT = sb.tile([1, R], BF16, name=f"ssrcT_L{L}h{h}", tag="ssrcT", bufs=2)
        nc.vector.tensor_copy(ssrcT[:, :], ps_s[:, :])

        # rho_b[j, i] = exp((a-1) * s_src_i): broadcast via PE then one ACT
        pb = ps_big.tile([128, R], F32, name=f"pb_L{L}h{h}", tag="ps_big")
        nc.tensor.matmul(pb[:, :], lhsT=ones_row[0:1, :], rhs=ssrcT[:, :],
                         start=True, stop=True)
        rho = sb.tile([128, R], BF16, name=f"rho_L{L}h{h}", tag="rho", bufs=2)
        nc.scalar.activation(rho[:, :], pb[:, :], AF.Exp, scale=ALPHA - 1.0)

        # g_aug = x.T @ WTaug -> [512, 257] (g | s_dst); payload rows scaled
        # by v = exp(s_dst): [g*v | v | w] with w = exp((a-1) s_dst)
        pay_t = dram_pay.tile([R, GW], BF16, name=f"pay_L{L}h{h}", tag="pay")
        pl = sb.tile([128, RB * GW], BF16, name=f"pl_L{L}h{h}", tag="pl", bufs=2)
        for ib in range(RB):
            pg = ps_big.tile([128, HID + 1], F32, name=f"pg_L{L}h{h}b{ib}", tag="ps_big")
            for fb in range(FB):
                nc.tensor.matmul(pg[:, :], lhsT=x_tiles[fb][:, ib * 128:(ib + 1) * 128],
                                 rhs=WTaug[fb][:, :], start=(fb == 0),
                                 stop=(fb == FB - 1))
            o = ib * GW
            vloc = sb.tile([128, 1], F32, name=f"vloc_L{L}h{h}b{ib}", tag="vloc",
                           bufs=2 * RB)
            nc.scalar.activation(vloc[:, :], pg[:, HID:HID + 1], AF.Exp)
            # g*v (ACT copy with per-partition scale), v, w columns
            nc.scalar.activation(pl[:, o:o + HID], pg[:, 0:HID], AF.Copy,
                                 scale=vloc[:, 0:1])
            nc.vector.tensor_copy(pl[:, o + HID:o + HID + 1], vloc[:, :])
            nc.scalar.activation(pl[:, o + HID + 1:o + HID + 2],
                                 pg[:, HID:HID + 1], AF.Exp, scale=ALPHA - 1.0)
        # one DMA: SBUF [p, (ib, c)] -> DRAM [(ib, p), c]
        nc.sync.dma_start(out=pay_t.rearrange("(ib p) c -> p ib c", p=128),
                          in_=pl.rearrange("p (ib c) -> p ib c", c=GW))

        if h == 0:
            # split gather: halves land earlier so attention starts sooner
            gat_a = dram_gat.tile([N // 2, GW], BF16, name=f"gatA_L{L}h{h}",
                                  tag="gat", addr_space="Shared")
            gat_b = dram_gat.tile([N // 2, GW], BF16, name=f"gatB_L{L}h{h}",
                                  tag="gat", addr_space="Shared")
            nc.gpsimd.collective_compute(
                "AllGather", OP.bypass, replica_groups=groups,
                ins=[pay_t[0:R // 2, :].opt()], outs=[gat_a.opt()],
            )
            nc.gpsimd.collective_compute(
                "AllGather", OP.bypass, replica_groups=groups,
                ins=[pay_t[R // 2:R, :].opt()], outs=[gat_b.opt()],
            )
            head_state.append(((gat_a, gat_b), rho))
        else:
            gat_t = dram_gat.tile([N, GW], BF16, name=f"gat_L{L}h{h}", tag="gat",
                                  addr_space="Shared")
            nc.gpsimd.collective_compute(
                "AllGather", OP.bypass, replica_groups=groups,
                ins=[pay_t.opt()], outs=[gat_t.opt()],
            )
            head_state.append((gat_t, rho))

    # ---- Phase B: attention per head ----
    # gf chunk position k holds global j-chunk 4*(k%8) + k//8 (gather-arrival
    # order); the host permutes the adjacency mask into the same order, so
    # everything below indexes by position k only.
    xgatT = []
    for hc in range(HC):
        xg = sb.tile([128, R], BF16, name=f"xgatT_L{L}c{hc}", tag="xgatT", bufs=HC)
        xgatT.append(xg)

    for h in range(HEADS):
        gat_t, rho = head_state[h]
        gf = sb.tile([128, JB * GW], BF16, name=f"gf_L{L}h{h}", tag="gf", bufs=2)
        gfr = gf.rearrange("p (j c) -> p j c", c=GW)
        gfb = gf.rearrange("p (b cc c) -> p b cc c", b=4, c=GW)
        if h == 0:
            gat_a, gat_b = gat_t
            for q, g_t in ((0, gat_a), (1, gat_b)):
                gr = g_t.rearrange("(cc l p) c -> p l cc c", p=128, l=2)
                for lp in range(2):
                    s = 2 * q + lp
                    eng = nc.sync if lp == 0 else nc.scalar
                    eng.dma_start(out=gfb[:, s:s + 1], in_=gr[:, lp:lp + 1])
        else:
            gr = gat_t.rearrange("(cc l p) c -> p l cc c", p=128, l=4)
            for lp in range(4):
                eng = nc.sync if lp % 2 == 0 else nc.scalar
                eng.dma_start(out=gfb[:, lp:lp + 1], in_=gr[:, lp:lp + 1])
        # w strip (fp32 per-partition scalars for the tensor_scalar ops)
        wst = sb.tile([128, JB], F32, name=f"wst_L{L}h{h}", tag="wst", bufs=2)
        for q in range(4):
            js = slice(q * JB // 4, (q + 1) * JB // 4)
            nc.vector.tensor_copy(wst[:, js], gfr[:, js, GW - 1])

        U = []
        for ib in range(RB):
            u = ps_acc.tile([128, HID + 1], F32, name=f"U_L{L}h{h}b{ib}", tag="ps_acc")
            U.append(u)

        NG = JB // QG
        for g in range(NG):
            # tv[:, cg*R:(cg+1)*R] = max(rho * w_k, 1) for the QG chunks,
            # then one wide mask multiply over all QG chunks at once
            tv = sb.tile([128, QG * R], BF16, name=f"tv_L{L}h{h}g{g}", tag="tv",
                         bufs=2)
            for cg in range(QG):
                k = g * QG + cg
                nc.vector.tensor_scalar(tv[:, cg * R:(cg + 1) * R], rho[:, :],
                                        wst[:, k:k + 1], 1.0, OP.mult, OP.max)
            pm = sb.tile([128, QG * R], BF16, name=f"pm_L{L}h{h}g{g}", tag="pm",
                         bufs=2)
            nc.vector.tensor_mul(pm[:, :], tv[:, :],
                                 mask_all[:, g * QG * R:(g + 1) * QG * R])
            for cg in range(QG):
                k = g * QG + cg
                o = k * GW
                for ib in range(RB):
                    nc.tensor.matmul(U[ib][:, :],
                                     lhsT=pm[:, cg * R + ib * 128:cg * R + (ib + 1) * 128],
                                     rhs=gf[:, o:o + HID + 1], start=(k == 0),
                                     stop=(k == JB - 1))

        for ib in range(RB):
            rcp = sb.tile([128, 1], F32, name=f"rcp_L{L}h{h}b{ib}", tag="rcp", bufs=2)
            nc.vector.reciprocal(rcp[:, :], U[ib][:, HID:HID + 1])
            # hn = U * (1/den) via ACT copy with per-partition scale
            hn = sb.tile([128, HID], BF16, name=f"hn_L{L}h{h}b{ib}", tag="hn", bufs=2)
            nc.scalar.activation(hn[:, :], U[ib][:, 0:HID], AF.Copy,
                                 scale=rcp[:, 0:1])
            # elu(x) = max(x, exp(min(x, 0)) - 1); exp(min(x,0)) = exp(-relu(-x))
            t1 = sb.tile([128, HID], BF16, name=f"t1_L{L}h{h}b{ib}", tag="t1", bufs=2)
            nc.scalar.activation(t1[:, :], hn[:, :], AF.Relu, scale=-1.0)
            t2 = sb.tile([128, HID], BF16, name=f"t2_L{L}h{h}b{ib}", tag="t2", bufs=2)
            nc.scalar.activation(t2[:, :], t1[:, :], AF.Exp, scale=-1.0)
            eo = sb.tile([128, HID], BF16, name=f"eo_L{L}h{h}b{ib}", tag="eo", bufs=2)
            nc.vector.scalar_tensor_tensor(eo[:, :], t2[:, :], -1.0, hn[:, :],
                                           OP.add, OP.max)
            for cb in range(2):
                pt = ps_sm.tile([128, 128], BF16, name=f"pt_L{L}h{h}b{ib}c{cb}",
                                tag="ps_sm")
                nc.tensor.transpose(pt[:, :], eo[:, cb * 128:(cb + 1) * 128],
                                    ident[:, :])
                if cb == 0:
                    nc.vector.tensor_copy(
                        xgatT[h * 2 + cb][:, ib * 128:(ib + 1) * 128], pt[:, :])
                else:
                    nc.scalar.copy(
                        xgatT[h * 2 + cb][:, ib * 128:(ib + 1) * 128], pt[:, :])
    return xgatT


def _build_program():
    nc = bacc.Bacc("TRN2", target_bir_lowering=False, debug=False,
                   num_devices=NCORES)

    xT_in = nc.dram_tensor("xT", [IN_DIM, R], BF16, kind="ExternalInput").ap()
    mask_in = nc.dram_tensor("mask", [N, R], BF16, kind="ExternalInput").ap()
    W1_in = nc.dram_tensor("W1", [HEADS, HID, IN_DIM], BF16, kind="ExternalInput").ap()
    W1T_in = nc.dram_tensor("W1T", [HEADS, IN_DIM, HID], BF16, kind="ExternalInput").ap()
    a1_in = nc.dram_tensor("a1p", [HEADS, HID, 2], BF16, kind="ExternalInput").ap()
    W2_in = nc.dram_tensor("W2", [HEADS, HID, IN_DIM], BF16, kind="ExternalInput").ap()
    W2T_in = nc.dram_tensor("W2T", [HEADS, IN_DIM, HID], BF16, kind="ExternalInput").ap()
    a2_in = nc.dram_tensor("a2p", [HEADS, HID, 2], BF16, kind="ExternalInput").ap()
    outwT_in = nc.dram_tensor("outwT", [HID * HEADS, IN_DIM], BF16,
                              kind="ExternalInput").ap()
    outb_in = nc.dram_tensor("outb", [IN_DIM, 1], F32, kind="ExternalInput").ap()
    out2wT_in = nc.dram_tensor("out2wT", [HID * HEADS, OUT_DIM], BF16,
                               kind="ExternalInput").ap()
    out2b_in = nc.dram_tensor("out2b", [OUT_DIM, 1], F32, kind="ExternalInput").ap()
    outT = nc.dram_tensor("outT", [OUT_DIM, R], F32, kind="ExternalOutput").ap()

    with tile.TileContext(nc) as tc:
        with tc.tile_pool(name="sb", bufs=1) as sb, \
             tc.tile_pool(name="ps_acc", bufs=RB, space="PSUM") as ps_acc, \
             tc.tile_pool(name="ps_big", bufs=2, space="PSUM") as ps_big, \
             tc.tile_pool(name="ps_sm", bufs=2, space="PSUM") as ps_sm, \
             tc.tile_pool(name="dram_pay", bufs=4, space="DRAM") as dram_pay, \
             tc.tile_pool(name="dram_gat", bufs=4, space="DRAM") as dram_gat:

            pools = dict(sb=sb, ps_acc=ps_acc, ps_big=ps_big, ps_sm=ps_sm,
                         dram_pay=dram_pay, dram_gat=dram_gat)

            ones_row = sb.tile([1, 128], BF16, name="ones_row", tag="ones_row", bufs=1)
            nc.vector.memset(ones_row[:, :], 1.0)

            # tiny warm-up AllGather: absorbs the cross-core entry barrier
            # (core launch skew) while the weight DMAs stream in
            warm_s = sb.tile([16, 2], BF16, name="warm_s", tag="warm", bufs=1)
            nc.vector.memset(warm_s[:, :], 1.0)
            warm_d = dram_pay.tile([16, 2], BF16, name="warm_d", tag="warm_d")
            nc.gpsimd.dma_start(out=warm_d[:, :], in_=warm_s[:, :])
            warm_g = dram_gat.tile([128, 2], BF16, name="warm_g", tag="warm_g",
                                   addr_space="Shared")
            nc.gpsimd.collective_compute(
                "AllGather", OP.bypass, replica_groups=[list(range(NCORES))],
                ins=[warm_d.opt()], outs=[warm_g.opt()],
            )

            # constants (identity uses gpsimd affine_select - after the
            # collective so it does not delay the barrier)
            ident = sb.tile([128, 128], BF16, name="ident", tag="ident", bufs=1)
            make_identity(nc, ident[:, :])
            pools["ident"] = ident
            pools["ones_row"] = ones_row

            # critical-path inputs on the sync queue (front of the line)
            x0 = []
            for fb in range(FB):
                x = sb.tile([128, R], BF16, name=f"x0_{fb}", tag="x0", bufs=FB)
                nc.sync.dma_start(out=x[:, :], in_=xT_in[fb * 128:(fb + 1) * 128, :])
                x0.append(x)

            # adjacency mask (multiplicative 0/1, PERMUTED to arrival order
            # host-side) on the gpsimd queue so it never delays weights
            mask_all = sb.tile([128, JB * R], BF16, name="mask_all", tag="mask",
                               bufs=1)
            for q in range(4):
                js = slice(q * JB // 4, (q + 1) * JB // 4)
                nc.gpsimd.dma_start(
                    out=mask_all.rearrange("p (j c) -> p j c", c=R)[:, js],
                    in_=mask_in.rearrange("(j p) c -> p j c", p=128)[:, js])

            # non-urgent weights ride the gpsimd queue behind the mask
            outw_t = []
            for hc in range(HC):
                w = sb.tile([128, IN_DIM], BF16, name=f"outw{hc}", tag="outw", bufs=HC)
                nc.gpsimd.dma_start(out=w[:, :],
                                    in_=outwT_in[hc * 128:(hc + 1) * 128, :])
                outw_t.append(w)
            out2w_t = []
            for hc in range(HC):
                w = sb.tile([128, OUT_DIM], BF16, name=f"out2w{hc}", tag="out2w",
                            bufs=HC)
                nc.gpsimd.dma_start(out=w[:, :],
                                    in_=out2wT_in[hc * 128:(hc + 1) * 128, :])
                out2w_t.append(w)
            outb_t = []
            for fb in range(FB):
                b = sb.tile([128, 1], F32, name=f"outb{fb}", tag="outb", bufs=FB)
                nc.gpsimd.dma_start(out=b[:, :], in_=outb_in[fb * 128:(fb + 1) * 128, :])
                outb_t.append(b)
            out2b_t = sb.tile([OUT_DIM, 1], F32, name="out2b", tag="out2b", bufs=1)
            nc.gpsimd.dma_start(out=out2b_t[:, :], in_=out2b_in[:, :])

            # ---- layer 1 ----
            xg1 = _build_layer(nc, tc, pools, x0, W1_in, W1T_in, a1_in, mask_all, 1)
            x1 = []
            for fb in range(FB):
                px = ps_big.tile([128, R], F32, name=f"px1_{fb}", tag="ps_big")
                for hc in range(HC):
                    nc.tensor.matmul(px[:, :], lhsT=outw_t[hc][:, fb * 128:(fb + 1) * 128],
                                     rhs=xg1[hc][:, :], start=(hc == 0),
                                     stop=(hc == HC - 1))
                x = sb.tile([128, R], BF16, name=f"x1_{fb}", tag="x1", bufs=FB)
                if fb % 2 == 0:
                    nc.vector.tensor_scalar(x[:, :], px[:, :], outb_t[fb][:, 0:1],
                                            None, OP.add)
                else:
                    nc.scalar.activation(x[:, :], px[:, :], AF.Identity,
                                         bias=outb_t[fb][:, 0:1])
                x1.append(x)

            # ---- layer 2 ----
            xg2 = _build_layer(nc, tc, pools, x1, W2_in, W2T_in, a2_in, mask_all, 2)
            po = ps_big.tile([OUT_DIM, R], F32, name="po", tag="ps_big")
            for hc in range(HC):
                nc.tensor.matmul(po[:, :], lhsT=out2w_t[hc][:, 0:OUT_DIM],
                                 rhs=xg2[hc][:, :], start=(hc == 0),
                                 stop=(hc == HC - 1))
            ot = sb.tile([OUT_DIM, R], F32, name="ot", tag="ot", bufs=1)
            nc.vector.tensor_scalar(ot[:, :], po[:, :], out2b_t[:, 0:1], None, OP.add)
            nc.sync.dma_start(out=outT[:, :], in_=ot[:, :])

    nc.compile()
    return nc


# gf/mask chunk position k holds global j-chunk _PERM[k] = 4*(k%8) + k//8
_PERM = [4 * (k % 8) + k // 8 for k in range(JB)]


def _host_shards(label_mat, W1, a1, W2, a2, out_w, out_b, out2_w, out2_b, adj):
    f32 = np.float32
    bf16 = ml_dtypes.bfloat16
    label_T = np.asarray(label_mat, f32).T.astype(bf16)                 # [768, N]
    adjT = np.asarray(adj).T.astype(bf16)                               # [N, N]
    # permute source-row blocks into gather-arrival order
    adjT_perm = np.ascontiguousarray(
        adjT.reshape(JB, 128, N)[_PERM].reshape(N, N))
    common = dict(
        W1=np.ascontiguousarray(np.asarray(W1, f32).astype(bf16)),
        W1T=np.ascontiguousarray(np.asarray(W1, f32).transpose(0, 2, 1).astype(bf16)),
        a1p=np.ascontiguousarray(np.asarray(a1, f32).reshape(HEADS, 2, HID)
                                 .transpose(0, 2, 1).astype(bf16)),
        W2=np.ascontiguousarray(np.asarray(W2, f32).astype(bf16)),
        W2T=np.ascontiguousarray(np.asarray(W2, f32).transpose(0, 2, 1).astype(bf16)),
        a2p=np.ascontiguousarray(np.asarray(a2, f32).reshape(HEADS, 2, HID)
                                 .transpose(0, 2, 1).astype(bf16)),
        outwT=np.ascontiguousarray(np.asarray(out_w, f32).T.astype(bf16)),
        outb=np.ascontiguousarray(np.asarray(out_b, f32).reshape(IN_DIM, 1)),
        out2wT=np.ascontiguousarray(np.asarray(out2_w, f32).T.astype(bf16)),
        out2b=np.ascontiguousarray(np.asarray(out2_b, f32).reshape(OUT_DIM, 1)),
    )
    in_maps = []
    for c in range(NCORES):
        sl = slice(c * R, (c + 1) * R)
        m = dict(common)
        m["xT"] = np.ascontiguousarray(label_T[:, sl])
        m["mask"] = np.ascontiguousarray(adjT_perm[:, sl])
        in_maps.append(m)
    return in_maps


def kernel(**inputs):
    global _nc_cache, last_exec_time_ns
    if _nc_cache is None:
        _nc_cache = _build_program()
    nc = _nc_cache
    in_maps = _host_shards(**inputs)
    trace = os.environ.get("GAT_TRACE", "0") == "1"
    res = run_bass_kernel_spmd(nc, in_maps, list(range(NCORES)), trace=trace)
    last_exec_time_ns = res.exec_time_ns
    out = np.empty((N, OUT_DIM), np.float32)
    for c in range(NCORES):
        out[c * R:(c + 1) * R, :] = np.asarray(res.results[c]["outT"]).T
    return out


# revision 16
# speedup vs baseline: 1.3916x; 1.0355x over previous
"""Two-layer GAT (N=4096, 4 heads, HID=256) on 8 TRN2 NeuronCores.

Sharding: each core owns N/8 = 512 destination rows of every N^2 attention
matrix. Weights are replicated. Per head we compute the local projection
g_shard = h_shard @ W.T on the owning core, then AllGather a packed
[512, 258] payload so every core has the full g for the attention matmul.

Softmax restructuring (the key to engine balance):
  exp(lrelu(x)) = max(exp(x), exp(a*x)) with x = s_i + t_j, so the masked
  unnormalized weight is  m * v_j * u_i * max(rho_i * w_j, 1)  with
  u = exp(s_src), v = exp(s_dst), rho = exp((a-1)s_src), w = exp((a-1)s_dst).
  - u_i is constant along j, so it cancels in the softmax normalization:
    never computed.
  - v_j is folded into the gathered payload on the OWNING core: the payload
    carries [g*v | v | w], so the attention matmul against columns [0:257]
    directly accumulates both the numerator and the denominator of the
    v-weighted softmax.
  - what remains per [128, 512] attention tile is
        pm = (rho_bcast * w_j  max  1) * mask
    i.e. ONE tensor_scalar (4x DVE mode: single-source bf16) plus a mask
    tensor_tensor which is batched over FOUR j-chunks at a time
    ([128, 2048]) to amortize instruction overhead. The adjacency mask is
    permuted host-side into gather-arrival order so those 4-chunk slices
    are contiguous and the same layout works for the split first gather.

Layout: attention tiles are [j=source (partition), i=dest (free)], so pm
tiles feed the PE matmul directly as lhsT; accumulation in fp32 PSUM with
the denominator riding in column 256. All matmul operands bf16.

Stall avoidance: a tiny warm-up AllGather is issued first so the
cross-core entry barrier (which absorbs core launch skew) overlaps the
initial weight DMAs; the 4 MB adjacency mask streams on the gpsimd DMA
queue; head 0's real gather is split in two so attention starts after
half the payload has arrived.
"""

import os

import numpy as np
import ml_dtypes

import concourse.bass as bass
import concourse.tile as tile
from concourse import bacc, mybir
from concourse.bass_utils import run_bass_kernel_spmd
from concourse.masks import make_identity

N, IN_DIM, HID, HEADS, OUT_DIM = 4096, 768, 256, 4, 32
ALPHA = 0.2
NCORES = 8
R = N // NCORES          # 512 rows per core
RB = R // 128            # 4 row blocks
FB = IN_DIM // 128       # 6 feature blocks
JB = N // 128            # 32 source chunks
QG = 8                   # j-chunks per mask-multiply group
HC = (HID * HEADS) // 128  # 8 concat-feature chunks
GW = HID + 2             # payload width: g*v (256) | v | w

F32 = mybir.dt.float32
BF16 = mybir.dt.bfloat16
AF = mybir.ActivationFunctionType
OP = mybir.AluOpType

last_exec_time_ns = None
_nc_cache = None


def _build_layer(nc, tc, pools, x_tiles, W_ap, WT_ap, ap_ap, mask_all, L):
    """One GAT layer. x_tiles: 6 SBUF tiles [128, R] bf16 (features x rows,
    feature-major). Returns 8 SBUF tiles [128, R] bf16 = concat-head
    activations transposed (x_gatT), elu applied."""
    sb = pools["sb"]
    ps_acc = pools["ps_acc"]
    ps_big = pools["ps_big"]
    ps_sm = pools["ps_sm"]
    dram_pay = pools["dram_pay"]
    dram_gat = pools["dram_gat"]
    ones_row = pools["ones_row"]
    ident = pools["ident"]

    groups = [list(range(NCORES))]

    head_state = []
    # ---- Phase A: per-head projection + payload + AllGather ----
    for h in range(HEADS):
        # weights for this head
        W_t = []
        for cc in range(2):
            wt = sb.tile([128, IN_DIM], BF16, name=f"W_L{L}h{h}c{cc}", tag="Wh", bufs=4)
            nc.sync.dma_start(out=wt[:, :], in_=W_ap[h, cc * 128:(cc + 1) * 128, :])
            W_t.append(wt)
        WTaug = []
        for fb in range(FB):
            wta = sb.tile([128, HID + 1], BF16, name=f"WTa_L{L}h{h}f{fb}", tag="WTaug",
                          bufs=2 * FB)
            nc.sync.dma_start(out=wta[:, 0:HID], in_=WT_ap[h, fb * 128:(fb + 1) * 128, :])
            WTaug.append(wta)
        a_t = []
        for cc in range(2):
            at = sb.tile([128, 2], BF16, name=f"a_L{L}h{h}c{cc}", tag="ah", bufs=4)
            nc.sync.dma_start(out=at[:, :], in_=ap_ap[h, cc * 128:(cc + 1) * 128, :])
            a_t.append(at)

        # w_eff[f, 0:2] = W.T @ [a_src | a_dst]  -> [768, 2] in 6 blocks
        weff = []
        for fb in range(FB):
            pw = ps_sm.tile([128, 2], F32, name=f"pw_L{L}h{h}f{fb}", tag="ps_sm")
            for cc in range(2):
                nc.tensor.matmul(pw[:, :], lhsT=W_t[cc][:, fb * 128:(fb + 1) * 128],
                                 rhs=a_t[cc][:, :], start=(cc == 0), stop=(cc == 1))
            wf = sb.tile([128, 2], BF16, name=f"weff_L{L}h{h}f{fb}", tag="weff",
                         bufs=2 * FB)
            nc.vector.tensor_copy(wf[:, :], pw[:, :])
            # dst half becomes column HID of the projection rhs
            nc.vector.tensor_copy(WTaug[fb][:, HID:HID + 1], wf[:, 1:2])
            weff.append(wf)

        # s_srcT [1, R] = w_eff_src.T @ x
        ps_s = ps_sm.tile([1, R], F32, name=f"ps_s_L{L}h{h}", tag="ps_sm")
        for fb in range(FB):
            nc.tensor.matmul(ps_s[:, :], lhsT=weff[fb][:, 0:1], rhs=x_tiles[fb][:, :],
                             start=(fb == 0), stop=(fb == FB - 1))
        ssrcT = sb.tile([1, R], BF16, name=f"ssrcT_L{L}h{h}", tag="ssrcT", bufs=2)
        nc.vector.tensor_copy(ssrcT[:, :], ps_s[:, :])

        # rho_b[j, i] = exp((a-1) * s_src_i): broadcast via PE then one ACT
        pb = ps_big.tile([128, R], F32, name=f"pb_L{L}h{h}", tag="ps_big")
        nc.tensor.matmul(pb[:, :], lhsT=ones_row[0:1, :], rhs=ssrcT[:, :],
                         start=True, stop=True)
        rho = sb.tile([128, R], BF16, name=f"rho_L{L}h{h}", tag="rho", bufs=2)
        nc.scalar.activation(rho[:, :], pb[:, :], AF.Exp, scale=ALPHA - 1.0)

        # g_aug = x.T @ WTaug -> [512, 257] (g | s_dst); payload rows scaled
        # by v = exp(s_dst): [g*v | v | w] with w = exp((a-1) s_dst)
        pay_t = dram_pay.tile([R, GW], BF16, name=f"pay_L{L}h{h}", tag="pay")
        pl = sb.tile([128, RB * GW], BF16, name=f"pl_L{L}h{h}", tag="pl", bufs=2)
        for ib in range(RB):
            pg = ps_big.tile([128, HID + 1], F32, name=f"pg_L{L}h{h}b{ib}", tag="ps_big")
            for fb in range(FB):
                nc.tensor.matmul(pg[:, :], lhsT=x_tiles[fb][:, ib * 128:(ib + 1) * 128],
                                 rhs=WTaug[fb][:, :], start=(fb == 0),
                                 stop=(fb == FB - 1))
            o = ib * GW
            vloc = sb.tile([128, 1], F32, name=f"vloc_L{L}h{h}b{ib}", tag="vloc",
                           bufs=2 * RB)
            nc.scalar.activation(vloc[:, :], pg[:, HID:HID + 1], AF.Exp)
            # g*v (ACT copy with per-partition scale), v, w columns
            nc.scalar.activation(pl[:, o:o + HID], pg[:, 0:HID], AF.Copy,
                                 scale=vloc[:, 0:1])
            nc.vector.tensor_copy(pl[:, o + HID:o + HID + 1], vloc[:, :])
            nc.scalar.activation(pl[:, o + HID + 1:o + HID + 2],
                                 pg[:, HID:HID + 1], AF.Exp, scale=ALPHA - 1.0)
        # one DMA: SBUF [p, (ib, c)] -> DRAM [(ib, p), c]
        nc.sync.dma_start(out=pay_t.rearrange("(ib p) c -> p ib c", p=128),
                          in_=pl.rearrange("p (ib c) -> p ib c", c=GW))

        gat_t = dram_gat.tile([N, GW], BF16, name=f"gat_L{L}h{h}", tag="gat",
                              addr_space="Shared")
        nc.gpsimd.collective_compute(
            "AllGather", OP.bypass, replica_groups=groups,
            ins=[pay_t.opt()], outs=[gat_t.opt()],
        )
        head_state.append((gat_t, rho))

    # ---- Phase B: attention per head ----
    # gf chunk position k holds global j-chunk 4*(k%8) + k//8 (gather-arrival
    # order); the host permutes the adjacency mask into the same order, so
    # everything below indexes by position k only.
    xgatT = []
    for hc in range(HC):
        xg = sb.tile([128, R], BF16, name=f"xgatT_L{L}c{hc}", tag="xgatT", bufs=HC)
        xgatT.append(xg)

    for h in range(HEADS):
        gat_t, rho = head_state[h]
        gf = sb.tile([128, JB * GW], BF16, name=f"gf_L{L}h{h}", tag="gf", bufs=2)
        gfr = gf.rearrange("p (j c) -> p j c", c=GW)
        gfb = gf.rearrange("p (b cc c) -> p b cc c", b=4, c=GW)
        gr = gat_t.rearrange("(cc l p) c -> p l cc c", p=128, l=4)
        for lp in range(4):
            eng = nc.sync if lp % 2 == 0 else nc.scalar
            eng.dma_start(out=gfb[:, lp:lp + 1], in_=gr[:, lp:lp + 1])
        # w strip (fp32 per-partition scalars for the tensor_scalar ops)
        wst = sb.tile([128, JB], F32, name=f"wst_L{L}h{h}", tag="wst", bufs=2)
        for q in range(4):
            js = slice(q * JB // 4, (q + 1) * JB // 4)
            nc.vector.tensor_copy(wst[:, js], gfr[:, js, GW - 1])

        U = []
        for ib in range(RB):
            u = ps_acc.tile([128, HID + 1], F32, name=f"U_L{L}h{h}b{ib}", tag="ps_acc")
            U.append(u)

        NG = JB // QG
        for g in range(NG):
            # tv[:, cg*R:(cg+1)*R] = max(rho * w_k, 1) for the QG chunks,
            # then one wide mask multiply over all QG chunks at once
            tv = sb.tile([128, QG * R], BF16, name=f"tv_L{L}h{h}g{g}", tag="tv",
                         bufs=2)
            for cg in range(QG):
                k = g * QG + cg
                nc.vector.tensor_scalar(tv[:, cg * R:(cg + 1) * R], rho[:, :],
                                        wst[:, k:k + 1], 1.0, OP.mult, OP.max)
            pm = sb.tile([128, QG * R], BF16, name=f"pm_L{L}h{h}g{g}", tag="pm",
                         bufs=2)
            nc.vector.tensor_mul(pm[:, :], tv[:, :],
                                 mask_all[:, g * QG * R:(g + 1) * QG * R])
            for cg in range(QG):
                k = g * QG + cg
                o = k * GW
                for ib in range(RB):
                    nc.tensor.matmul(U[ib][:, :],
                                     lhsT=pm[:, cg * R + ib * 128:cg * R + (ib + 1) * 128],
                                     rhs=gf[:, o:o + HID + 1], start=(k == 0),
                                     stop=(k == JB - 1))

        for ib in range(RB):
            rcp = sb.tile([128, 1], F32, name=f"rcp_L{L}h{h}b{ib}", tag="rcp", bufs=2)
            nc.vector.reciprocal(rcp[:, :], U[ib][:, HID:HID + 1])
            # hn = U * (1/den) via ACT copy with per-partition scale
            hn = sb.tile([128, HID], BF16, name=f"hn_L{L}h{h}b{ib}", tag="hn", bufs=2)
            nc.scalar.activation(hn[:, :], U[ib][:, 0:HID], AF.Copy,
                                 scale=rcp[:, 0:1])
            # elu(x) = max(x, exp(min(x, 0)) - 1); exp(min(x,0)) = exp(-relu(-x))
            t1 = sb.tile([128, HID], BF16, name=f"t1_L{L}h{h}b{ib}", tag="t1", bufs=2)
            nc.scalar.activation(t1[:, :], hn[:, :], AF.Relu, scale=-1.0)
            t2 = sb.tile([128, HID], BF16, name=f"t2_L{L}h{h}b{ib}", tag="t2", bufs=2)
            nc.scalar.activation(t2[:, :], t1[:, :], AF.Exp, scale=-1.0)
            eo = sb.tile([128, HID], BF16, name=f"eo_L{L}h{h}b{ib}", tag="eo", bufs=2)
            nc.vector.scalar_tensor_tensor(eo[:, :], t2[:, :], -1.0, hn[:, :],
                                           OP.add, OP.max)
            for cb in range(2):
                pt = ps_sm.tile([128, 128], BF16, name=f"pt_L{L}h{h}b{ib}c{cb}",
                                tag="ps_sm")
                nc.tensor.transpose(pt[:, :], eo[:, cb * 128:(cb + 1) * 128],
                                    ident[:, :])
                nc.scalar.copy(
                    xgatT[h * 2 + cb][:, ib * 128:(ib + 1) * 128], pt[:, :])
    return xgatT


def _build_program():
    nc = bacc.Bacc("TRN2", target_bir_lowering=False, debug=False,
                   num_devices=NCORES)

    xT_in = nc.dram_tensor("xT", [IN_DIM, R], BF16, kind="ExternalInput").ap()
    mask_in = nc.dram_tensor("mask", [N, R], BF16, kind="ExternalInput").ap()
    W1_in = nc.dram_tensor("W1", [HEADS, HID, IN_DIM], BF16, kind="ExternalInput").ap()
    W1T_in = nc.dram_tensor("W1T", [HEADS, IN_DIM, HID], BF16, kind="ExternalInput").ap()
    a1_in = nc.dram_tensor("a1p", [HEADS, HID, 2], BF16, kind="ExternalInput").ap()
    W2_in = nc.dram_tensor("W2", [HEADS, HID, IN_DIM], BF16, kind="ExternalInput").ap()
    W2T_in = nc.dram_tensor("W2T", [HEADS, IN_DIM, HID], BF16, kind="ExternalInput").ap()
    a2_in = nc.dram_tensor("a2p", [HEADS, HID, 2], BF16, kind="ExternalInput").ap()
    outwT_in = nc.dram_tensor("outwT", [HID * HEADS, IN_DIM], BF16,
                              kind="ExternalInput").ap()
    outb_in = nc.dram_tensor("outb", [IN_DIM, 1], F32, kind="ExternalInput").ap()
    out2wT_in = nc.dram_tensor("out2wT", [HID * HEADS, OUT_DIM], BF16,
                               kind="ExternalInput").ap()
    out2b_in = nc.dram_tensor("out2b", [OUT_DIM, 1], F32, kind="ExternalInput").ap()
    outT = nc.dram_tensor("outT", [OUT_DIM, R], F32, kind="ExternalOutput").ap()

    with tile.TileContext(nc) as tc:
        with tc.tile_pool(name="sb", bufs=1) as sb, \
             tc.tile_pool(name="ps_acc", bufs=RB, space="PSUM") as ps_acc, \
             tc.tile_pool(name="ps_big", bufs=2, space="PSUM") as ps_big, \
             tc.tile_pool(name="ps_sm", bufs=2, space="PSUM") as ps_sm, \
             tc.tile_pool(name="dram_pay", bufs=4, space="DRAM") as dram_pay, \
             tc.tile_pool(name="dram_gat", bufs=4, space="DRAM") as dram_gat:

            pools = dict(sb=sb, ps_acc=ps_acc, ps_big=ps_big, ps_sm=ps_sm,
                         dram_pay=dram_pay, dram_gat=dram_gat)

            ones_row = sb.tile([1, 128], BF16, name="ones_row", tag="ones_row", bufs=1)
            nc.vector.memset(ones_row[:, :], 1.0)

            # constants (identity uses gpsimd affine_select - after the
            # collective so it does not delay the barrier)
            ident = sb.tile([128, 128], BF16, name="ident", tag="ident", bufs=1)
            make_identity(nc, ident[:, :])
            pools["ident"] = ident
            pools["ones_row"] = ones_row

            # critical-path inputs on the sync queue (front of the line)
            x0 = []
            for fb in range(FB):
                x = sb.tile([128, R], BF16, name=f"x0_{fb}", tag="x0", bufs=FB)
                nc.sync.dma_start(out=x[:, :], in_=xT_in[fb * 128:(fb + 1) * 128, :])
                x0.append(x)

            # adjacency mask (multiplicative 0/1, PERMUTED to arrival order
            # host-side) on the gpsimd queue so it never delays weights
            mask_all = sb.tile([128, JB * R], BF16, name="mask_all", tag="mask",
                               bufs=1)
            for q in range(4):
                js = slice(q * JB // 4, (q + 1) * JB // 4)
                nc.gpsimd.dma_start(
                    out=mask_all.rearrange("p (j c) -> p j c", c=R)[:, js],
                    in_=mask_in.rearrange("(j p) c -> p j c", p=128)[:, js])

            # non-urgent weights ride the gpsimd queue behind the mask
            outw_t = []
            for hc in range(HC):
                w = sb.tile([128, IN_DIM], BF16, name=f"outw{hc}", tag="outw", bufs=HC)
                nc.gpsimd.dma_start(out=w[:, :],
                                    in_=outwT_in[hc * 128:(hc + 1) * 128, :])
                outw_t.append(w)
            out2w_t = []
            for hc in range(HC):
                w = sb.tile([128, OUT_DIM], BF16, name=f"out2w{hc}", tag="out2w",
                            bufs=HC)
                nc.gpsimd.dma_start(out=w[:, :],
                                    in_=out2wT_in[hc * 128:(hc + 1) * 128, :])
                out2w_t.append(w)
            outb_t = []
            for fb in range(FB):
                b = sb.tile([128, 1], F32, name=f"outb{fb}", tag="outb", bufs=FB)
                nc.gpsimd.dma_start(out=b[:, :], in_=outb_in[fb * 128:(fb + 1) * 128, :])
                outb_t.append(b)
            out2b_t = sb.tile([OUT_DIM, 1], F32, name="out2b", tag="out2b", bufs=1)
            nc.gpsimd.dma_start(out=out2b_t[:, :], in_=out2b_in[:, :])

            # ---- layer 1 ----
            xg1 = _build_layer(nc, tc, pools, x0, W1_in, W1T_in, a1_in, mask_all, 1)
            x1 = []
            for fb in range(FB):
                px = ps_big.tile([128, R], F32, name=f"px1_{fb}", tag="ps_big")
                for hc in range(HC):
                    nc.tensor.matmul(px[:, :], lhsT=outw_t[hc][:, fb * 128:(fb + 1) * 128],
                                     rhs=xg1[hc][:, :], start=(hc == 0),
                                     stop=(hc == HC - 1))
                x = sb.tile([128, R], BF16, name=f"x1_{fb}", tag="x1", bufs=FB)
                nc.scalar.activation(x[:, :], px[:, :], AF.Identity,
                                     bias=outb_t[fb][:, 0:1])
                x1.append(x)

            # ---- layer 2 ----
            xg2 = _build_layer(nc, tc, pools, x1, W2_in, W2T_in, a2_in, mask_all, 2)
            po = ps_big.tile([OUT_DIM, R], F32, name="po", tag="ps_big")
            for hc in range(HC):
                nc.tensor.matmul(po[:, :], lhsT=out2w_t[hc][:, 0:OUT_DIM],
                                 rhs=xg2[hc][:, :], start=(hc == 0),
                                 stop=(hc == HC - 1))
            ot = sb.tile([OUT_DIM, R], F32, name="ot", tag="ot", bufs=1)
            nc.vector.tensor_scalar(ot[:, :], po[:, :], out2b_t[:, 0:1], None, OP.add)
            nc.sync.dma_start(out=outT[:, :], in_=ot[:, :])

    nc.compile()
    return nc


# gf/mask chunk position k holds global j-chunk _PERM[k] = 4*(k%8) + k//8
_PERM = [4 * (k % 8) + k // 8 for k in range(JB)]


def _host_shards(label_mat, W1, a1, W2, a2, out_w, out_b, out2_w, out2_b, adj):
    f32 = np.float32
    bf16 = ml_dtypes.bfloat16
    label_T = np.asarray(label_mat, f32).T.astype(bf16)                 # [768, N]
    adjT = np.asarray(adj).T.astype(bf16)                               # [N, N]
    # permute source-row blocks into gather-arrival order
    adjT_perm = np.ascontiguousarray(
        adjT.reshape(JB, 128, N)[_PERM].reshape(N, N))
    common = dict(
        W1=np.ascontiguousarray(np.asarray(W1, f32).astype(bf16)),
        W1T=np.ascontiguousarray(np.asarray(W1, f32).transpose(0, 2, 1).astype(bf16)),
        a1p=np.ascontiguousarray(np.asarray(a1, f32).reshape(HEADS, 2, HID)
                                 .transpose(0, 2, 1).astype(bf16)),
        W2=np.ascontiguousarray(np.asarray(W2, f32).astype(bf16)),
        W2T=np.ascontiguousarray(np.asarray(W2, f32).transpose(0, 2, 1).astype(bf16)),
        a2p=np.ascontiguousarray(np.asarray(a2, f32).reshape(HEADS, 2, HID)
                                 .transpose(0, 2, 1).astype(bf16)),
        outwT=np.ascontiguousarray(np.asarray(out_w, f32).T.astype(bf16)),
        outb=np.ascontiguousarray(np.asarray(out_b, f32).reshape(IN_DIM, 1)),
        out2wT=np.ascontiguousarray(np.asarray(out2_w, f32).T.astype(bf16)),
        out2b=np.ascontiguousarray(np.asarray(out2_b, f32).reshape(OUT_DIM, 1)),
    )
    in_maps = []
    for c in range(NCORES):
        sl = slice(c * R, (c + 1) * R)
        m = dict(common)
        m["xT"] = np.ascontiguousarray(label_T[:, sl])
        m["mask"] = np.ascontiguousarray(adjT_perm[:, sl])
        in_maps.append(m)
    return in_maps


def kernel(**inputs):
    global _nc_cache, last_exec_time_ns
    if _nc_cache is None:
        _nc_cache = _build_program()
    nc = _nc_cache
    in_maps = _host_shards(**inputs)
    trace = os.environ.get("GAT_TRACE", "0") == "1"
    res = run_bass_kernel_spmd(nc, in_maps, list(range(NCORES)), trace=trace)
    last_exec_time_ns = res.exec_time_ns
    out = np.empty((N, OUT_DIM), np.float32)
    for c in range(NCORES):
        out[c * R:(c + 1) * R, :] = np.asarray(res.results[c]["outT"]).T
    return out
